# revision 17
# baseline (speedup 1.0000x reference)
"""MaskLinear raw-bass kernel (no TileContext) for 8x TRN2 NeuronCores.

out[m,d] = sum_n w[n]*mask[m,n]*x[n,d] + bias, decomposed as
  mask = 0.5 + c,  c in [-0.5, 0.5)  (c quantized to fp8 e4m3)
  out  = [sum_n w*c_mn*x_nd]  (device)  +  0.5*(w @ x)[d] + bias  (host)

Device per core (N/8 = 12500 rows, padded to 98*128 = 12544):
  - "wg" f16 [128, 34*289]: per chunk [c8 (64 fp8 as 32 f16 cols) | w*256 |
    x(256)] for the 4 fp8-mul groups. The c8 region is read on device via a
    size-changing f16->f8 AP bitcast, so each group's single DMA carries
    masks+weight+x together.
  - "wxb" fp8 [128, 64*320]: per chunk [wm8(64) | x8(256)] for 64 chunks,
    wm8 = e4m3(w*256*c) prefolded on host -> fully-fp8 matmuls (e4m3 of the
    product has the same relative error as e4m3(c)*w) at half the traffic.
  - Two HW DGE rings (sync/scalar), symmetric queues [small fp8-mul group,
    big fully-fp8 half, small fp8-mul group] so the heavy PE work lands
    mid-stream and only a tiny group trails; a cold-start flip of either
    ring degrades gracefully. All triggers are enqueued upfront; each ring
    round-robins ~2 in-flight DMAs, so queue order ~= completion order.
  - DVE: wm = c8 * w (fp8 x f16 broadcast mul -> f16) per fp8 group.
  - PE: per chunk pair, col-tiled matmuls accumulate psum[0:64]/[64:128].
  - copy psum -> sbuf f16, DMA out [128, 256] f16.
Host: sum partials over cores, fold halves, /256, + 0.5*(w@x) + bias.
"""

import numpy as np
import ml_dtypes

import concourse.bacc as bacc
import concourse.mybir as mybir
from concourse.bass_utils import run_bass_kernel_spmd

N_CORES = 8
N = 100000
D = 256
M = 64
NS = N // N_CORES
CHUNK = 128
C = -(-NS // CHUNK)            # 98
NP = C * CHUNK                 # 12544
CC = M // 2                    # 32 f16 cols holding 64 fp8 c values
WGW = CC + 1 + D               # 289 cols per chunk in wg
WBW = M + D                    # 320 fp8 cols per chunk in wb8
WSCALE = 256.0

# fp8-mul groups in consumption order, and their ring (A=sync, B=scalar).
FP8 = [(14, "A"), (12, "B"), (4, "A"), (4, "B")]
PREFOLD = [(32, "A"), (32, "B")]  # fully-fp8 tail: [wm8 | x8], no DVE mul
CA = sum(g for g, _ in FP8)    # 70
CB = sum(g for g, _ in PREFOLD)  # 28
assert CA + CB == C
assert all(g % 2 == 0 for g, _ in FP8 + PREFOLD)

_STATE = {}

f32 = mybir.dt.float32
f16 = mybir.dt.float16
f8 = mybir.dt.float8e4


def _build_nc():
    nc = bacc.Bacc("TRN2", target_bir_lowering=False, debug=False,
                   num_devices=N_CORES)
    wg = nc.dram_tensor("wg", [CHUNK, CA * WGW], f16, kind="ExternalInput")
    wxb = nc.dram_tensor("wxb", [CHUNK, CB * WBW], f8, kind="ExternalInput")
    out = nc.dram_tensor("out", [CHUNK, D], f16, kind="ExternalOutput")

    ctx = nc.ctx
    ngrp = len(FP8) + len(PREFOLD)
    gsem = [ctx.enter_context(nc.semaphore(f"gsem{j}")) for j in range(ngrp)]
    osem = ctx.enter_context(nc.semaphore("osem"))
    dsem = ctx.enter_context(nc.semaphore("dsem"))
    psem = ctx.enter_context(nc.semaphore("psem"))
    msem = ctx.enter_context(nc.semaphore("msem"))
    sb_wg = ctx.enter_context(nc.sbuf_tensor("sb_wg", [CHUNK, CA * WGW], f16))
    sb_wxb = ctx.enter_context(nc.sbuf_tensor("sb_wxb", [CHUNK, CB * WBW], f8))
    sb_wm = ctx.enter_context(nc.sbuf_tensor("sb_wm", [CHUNK, CA * M], f16))
    sb_o = ctx.enter_context(nc.sbuf_tensor("sb_o", [CHUNK, D], f16))
    psum = ctx.enter_context(nc.psum_tensor("ps", [CHUNK, D], f32))

    baseA = np.cumsum([0] + [g for g, _ in FP8])[:-1].tolist()
    baseB = np.cumsum([0] + [g for g, _ in PREFOLD])[:-1].tolist()
    eng = {"A": nc.sync, "B": nc.scalar}

    # --- DMA triggers, all upfront; per-ring queue order puts the big
    # fully-fp8 halves mid-stream so PE work spreads across the stream ---
    QUEUE = {"A": [("fp8", 0), ("wb", 0), ("fp8", 2)],
             "B": [("fp8", 1), ("wb", 1), ("fp8", 3)]}
    for ring in ("A", "B"):
        for kind, j in QUEUE[ring]:
            if kind == "fp8":
                g = FP8[j][0]
                b0 = baseA[j]
                eng[ring].dma_start(
                    sb_wg[:, b0 * WGW:(b0 + g) * WGW],
                    wg[:, b0 * WGW:(b0 + g) * WGW],
                ).then_inc(gsem[j], 16)
            else:
                g = PREFOLD[j][0]
                b0 = baseB[j]
                eng[ring].dma_start(
                    sb_wxb[:, b0 * WBW:(b0 + g) * WBW],
                    wxb[:, b0 * WBW:(b0 + g) * WBW],
                ).then_inc(gsem[len(FP8) + j], 16)

    # --- DVE muls per fp8 group; PE consumes fp8 0..3, then WB, then the
    # small fp8 group 4 last (it is the final DMA to land) ---
    pair = 0
    NPAIR = C // 2
    mmB = None

    nmul = [0]

    def fp8_group(j, last, mul_eng=None, mul_sem=None, mul_thr=None):
        nonlocal pair, mmB
        g = FP8[j][0]
        b0 = baseA[j]
        me = mul_eng or nc.vector
        ms = mul_sem or dsem
        me.wait_ge(gsem[j], 16)
        grp = sb_wg[:, b0 * WGW:(b0 + g) * WGW].rearrange(
            "p (b j) -> p b j", b=g)
        c3 = grp[:, :, 0:CC].bitcast(f8)          # [128, g, 64] fp8
        w3 = grp[:, :, CC:CC + 1]                 # [128, g, 1] f16
        wm3 = sb_wm[:, b0 * M:(b0 + g) * M].rearrange("p (b j) -> p b j", b=g)
        me.tensor_mul(
            wm3, c3, w3.broadcast_to((CHUNK, g, M))
        ).then_inc(ms, 1)
        if ms is dsem:
            nmul[0] += 1
        nc.tensor.wait_ge(ms, mul_thr if mul_thr is not None else nmul[0])
        for b in range(0, g, 2):
            cA = b0 + b
            cB = b0 + b + 1
            nc.tensor.matmul(
                psum[0:M, :],
                sb_wm[:, cA * M:(cA + 1) * M],
                sb_wg[:, cA * WGW + CC + 1:(cA + 1) * WGW],
                start=(pair == 0), stop=(last and pair == NPAIR - 1),
                tile_position=(0, 0),
            )
            mmB = nc.tensor.matmul(
                psum[M:2 * M, :],
                sb_wm[:, cB * M:(cB + 1) * M],
                sb_wg[:, cB * WGW + CC + 1:(cB + 1) * WGW],
                start=(pair == 0), stop=(last and pair == NPAIR - 1),
                tile_position=(0, M),
            )
            pair += 1

    fp8_group(0, last=False)
    fp8_group(1, last=False)
    for j, (g, _) in enumerate(PREFOLD):
        b0 = baseB[j]
        nc.tensor.wait_ge(gsem[len(FP8) + j], 16)
        for b in range(0, g, 2):
            cA = b0 + b
            cB = b0 + b + 1
            nc.tensor.matmul(
                psum[0:M, :],
                sb_wxb[:, cA * WBW:cA * WBW + M],
                sb_wxb[:, cA * WBW + M:(cA + 1) * WBW],
                start=False, stop=False,
                tile_position=(0, 0),
            )
            mmB = nc.tensor.matmul(
                psum[M:2 * M, :],
                sb_wxb[:, cB * WBW:cB * WBW + M],
                sb_wxb[:, cB * WBW + M:(cB + 1) * WBW],
                start=False, stop=False,
                tile_position=(0, M),
            )
            pair += 1
    fp8_group(2, last=False)
    fp8_group(3, last=True, mul_eng=nc.gpsimd, mul_sem=msem, mul_thr=1)
    assert pair == NPAIR
    mmB.then_inc(psem, 1)

    # Keep the PE sequencer hot through the copy/out tail.
    ps_j = ctx.enter_context(nc.psum_tensor("psj", [M, D], f32))
    for k in range(10):
        nc.tensor.matmul(
            ps_j[:, :],
            sb_wm[:, 0:M],
            sb_wg[:, CC + 1:CC + 1 + D],
            start=True, stop=True,
            tile_position=(0, 0),
        )

    # --- tail: copy psum -> sbuf f16, DMA out, wait completion on SP ---
    nc.vector.wait_ge(psem, 1)
    nc.vector.tensor_copy(sb_o[:, :], psum[:, :]).then_inc(psem, 1)
    nc.sync.wait_ge(psem, 2)
    nc.sync.dma_start(out[:, :], sb_o[:, :]).then_inc(osem, 16)
    nc.sync.wait_ge(osem, 16)

    nc.compile()
    return nc


def _get_nc():
    if "nc" not in _STATE:
        _STATE["nc"] = _build_nc()
    return _STATE["nc"]


def _shard_inputs(x, masks, weight):
    x = np.asarray(x, dtype=np.float32)
    masks = np.asarray(masks, dtype=np.float32)
    weight = np.asarray(weight, dtype=np.float32)
    f8np = np.dtype(ml_dtypes.float8_e4m3fn)

    in_maps = []
    for s in range(N_CORES):
        lo = s * NS
        hi = lo + NS
        xs = np.zeros((NP, D), np.float32)
        xs[:NS] = x[lo:hi]
        ws = np.zeros(NP, np.float32)
        ws[:NS] = weight[lo:hi] * WSCALE
        cs = np.zeros((NP, M), np.float32)
        cs[:NS] = masks[:, lo:hi].T - 0.5
        c8q = np.ascontiguousarray(cs.astype(f8np))

        NA = CA * CHUNK
        # wg: [c8 bytes | w | x] per chunk for the fp8 groups
        wgs = np.empty((NA, WGW), np.float16)
        wgs[:, :CC] = c8q[:NA].view(np.float16)
        wgs[:, CC] = ws[:NA].astype(np.float16)
        wgs[:, CC + 1:] = xs[:NA].astype(np.float16)
        wgp = wgs.reshape(CA, CHUNK, WGW).transpose(1, 0, 2).reshape(
            CHUNK, CA * WGW)
        # wb8: fully-fp8 tail chunks [wm8 | x8]; wm8 folds w*256*c in one
        # e4m3 quantization (same relative error as e4m3(c)*w).
        wxbs = np.empty((CB * CHUNK, WBW), f8np)
        wxbs[:, :M] = (ws[NA:, None] * cs[NA:]).astype(f8np)
        wxbs[:, M:] = xs[NA:].astype(f8np)
        wxbp = wxbs.reshape(CB, CHUNK, WBW).transpose(1, 0, 2).reshape(
            CHUNK, CB * WBW)
        in_maps.append({"wg": wgp, "wxb": wxbp})
    return in_maps


def _run(x, masks, weight, bias, **run_kwargs):
    in_maps = _shard_inputs(x, masks, weight)
    try:
        res = run_bass_kernel_spmd(
            _get_nc(), in_maps, core_ids=list(range(N_CORES)), **run_kwargs
        )
    except Exception:
        res = run_bass_kernel_spmd(
            _get_nc(), in_maps, core_ids=list(range(N_CORES)), **run_kwargs
        )
    parts = np.stack([np.asarray(r["out"], dtype=np.float32)
                      for r in res.results])       # [8, 128, 256]
    full = parts.sum(axis=0)
    full = (full[:M] + full[M:]) * np.float32(1.0 / WSCALE)
    w32 = np.asarray(weight, np.float32)
    x32 = np.asarray(x, np.float32)
    rank1 = 0.5 * (w32 @ x32)                      # [D]
    out = full + rank1[None, :] + np.asarray(bias, np.float32)
    return out.astype(np.float32), res


def kernel(x, masks, weight, bias):
    out, _ = _run(x, masks, weight, bias)
    return out
